# revision 20
# baseline (speedup 1.0000x reference)
"""Bidirectional chamfer loss kernel for Trainium2 (8 NeuronCores).

Problem (hardcoded): B=2 batches, V1=8192 gt points, V2=8192 pred points, 3D.
  d2[b,i,j] = max(0, |xp_i|^2 + |gt_j|^2 - 2 xp_i.gt_j),  xp = x_pred * mask
  loss_pred2gt[b,i] = sqrt(min_j d2) * 100
  loss_gt2pred[b,j] = sqrt(min_i d2) * 100
  loss_conf = (loss_pred2gt * conf - ln(conf)) * mask ; loss_pred2gt *= mask

Sharding: 8 cores = 2 batches x 4 V2-slices (2048 preds/core vs full 8192 gt).
Each core computes row mins (pred2gt) for its pred slice exactly, and a
partial col min (gt2pred) over its preds; the host combines partials with
np.maximum on -d2 (exact).

Host-side compaction: masked preds collapse to the origin and their
pred2gt outputs are zeroed by the mask anyway, so the host keeps only
unmasked preds (plus origin padding, which is idempotent for gt2pred --
every slice retains its masked-at-origin points) and pads to a multiple
of 128. For ~80% keep rate this drops npt from 16 to 13 tiles.

Device kernel (per core, SPMD):
  PE matmul cost is N moving columns regardless of contraction depth K<=128,
  so the fp16 hi/lo split (A_hi.G_hi + A_lo.G_hi + A_hi.G_lo) is packed
  into ONE K=15 matmul -- fp32-grade d2 at fp16 matmul cost. The A side is
  negated so the matmul yields -d2 and every fold is a MAX.

  Per pred tile (128 preds x full 8192 gt): 16 N=512 matmuls into 4
  [128,2048] PSUM tiles; ScalarE downconverts each once into one
  contiguous [128,8192] fp16 buffer (1 elem/cycle/lane, the drain floor).
  DVE then runs the fewest possible fp16 2x-mode TTs: ONE full-width
  column fold into colacc (pair-seeded at pt==1, so no seed copy), and
  two per-pt row halving levels (8192->4096->2048) into a slot of a
  [128, npt*2048] buffer; the remaining halving levels run BATCHED
  across all pred tiles as in-place 3D-AP TTs, ending in one batched
  [128,npt,128] TensorReduce. Both paths sit at the DVE structural
  floor (12 col TTs = information-theoretic minimum for 13 tiles; the
  cascade consumes 4 fp16/cycle/lane with batch-amortized init cost).

  gt2pred finish: gpsimd.partition_all_reduce(max) on the double-buffered
  colacc -- runs on the otherwise-idle GPSIMD and overlaps the next
  pass's main loop, costing the DVE nothing (the former PE-transpose +
  DVE-reduce tail cost ~10us of DVE).

  The device returns RAW -d2 row/col maxima; sqrt, *100, mask/confidence
  weighting, ln(conf), and scatter back to original pred positions all
  happen on the host (cheap numpy on 16K values) -- no activations on
  device at all, so no activation-table loads.

  The `repeat` build parameter wraps the ENTIRE body (input DMA, main
  loop, transpose finish, output DMA) so the work-scaling timing harness
  measures the full per-pass device time.
"""

import numpy as np

B = 2
V1 = 8192  # gt points
V2 = 8192  # pred points (total)
N_CORES = 8
SLICES = N_CORES // B  # V2-slices per batch
V2C = V2 // SLICES  # pred points per core

_BUILT = {}


def _build_v3(v1, v2c, repeat=1, mmw=512):
    import concourse.tile as tile
    from concourse import bacc, bass_isa, mybir

    f32 = mybir.dt.float32
    f16 = mybir.dt.float16
    MAX = mybir.AluOpType.max
    X = mybir.AxisListType.X

    npt = v2c // 128  # pred tiles
    W = min(2048, v1)  # gt group width: one PSUM tile, one ScalarE downconvert
    ng = v1 // W  # gt groups
    ngt = v1 // 128  # gt output tiles (transpose finish)
    nq = W // 512  # [128,512] transpose-output tiles per group
    S = v2c + v1

    nc = bacc.Bacc()
    ag_in = nc.dram_tensor("ag", [15, S], f16, kind="ExternalInput")
    o_all = nc.dram_tensor("o_all", [128, npt], f32, kind="ExternalOutput")
    g2p_out = nc.dram_tensor("g2p", [1, v1], f16, kind="ExternalOutput")

    CW = ng * W  # full gt width (8192): one col accumulator, one col TT
    HW2 = CW // 4  # per-pt cascade handoff width (2048)

    with tile.TileContext(nc) as tc:
        with (
            tc.tile_pool(name="persist", bufs=1) as P,
            tc.tile_pool(name="s16p", bufs=3) as S16P,
            tc.tile_pool(name="hp", bufs=2) as HP,
            tc.tile_pool(name="colp", bufs=2) as COLP,
            tc.tile_pool(name="mmps", bufs=2, space="PSUM") as MMPS,
        ):
            AG = P.tile([15, S], f16, tag="AG")
            A = AG[:, 0:v2c]
            G = AG[:, v2c:S]
            h2 = P.tile([128, npt * HW2], f16, tag="h2")
            p2g_min = P.tile([128, npt], f32, tag="p2gmin")
            g2p_all = P.tile([128, CW], f16, tag="g2pall")

            for _ in range(repeat):
                colacc = COLP.tile([128, CW], f16, tag="colacc")
                nc.sync.dma_start(AG[:], ag_in[:, :])

                # ---- main loop ----
                # All 4 gt-group PSUM tiles drain into one contiguous
                # [128, 8192] fp16 buffer, so the column fold is ONE
                # full-width TT and the row path is a halving cascade of
                # full-width TTs -- fewest possible DVE instructions.
                prev_big = None
                for pt in range(npt):
                    lhsT = A[:, pt * 128 : (pt + 1) * 128]
                    big = S16P.tile([128, CW], f16, tag="s16")
                    for g in range(ng):
                        ps = MMPS.tile([128, W], f32, tag="mm")
                        for i in range(W // mmw):
                            nc.tensor.matmul(
                                ps[:, i * mmw : (i + 1) * mmw],
                                lhsT,
                                G[:, g * W + i * mmw : g * W + (i + 1) * mmw],
                                start=True,
                                stop=True,
                            )
                        nc.scalar.copy(big[:, g * W : (g + 1) * W], ps[:])
                    # column fold: pair-seed at pt==1, plain fold after
                    if pt == 1:
                        nc.vector.tensor_tensor(
                            colacc[:], prev_big[:], big[:], op=MAX
                        )
                    elif pt > 1:
                        nc.vector.tensor_tensor(
                            colacc[:], colacc[:], big[:], op=MAX
                        )
                    prev_big = big
                    # row path: two halving levels per pred tile
                    # (CW -> CW/2 -> CW/4 into this pt's h2 slot); the
                    # remaining levels run batched across all pred tiles
                    # after the loop (fewer DVE instructions, same work).
                    h = HP.tile([128, CW // 2], f16, tag="h")
                    nc.vector.tensor_tensor(
                        h[:], big[:, 0 : CW // 2], big[:, CW // 2 : CW], op=MAX
                    )
                    nc.vector.tensor_tensor(
                        h2[:, pt * HW2 : pt * HW2 + HW2],
                        h[:, 0:HW2],
                        h[:, HW2 : 2 * HW2],
                        op=MAX,
                    )
                if npt == 1:
                    nc.vector.tensor_copy(colacc[:], prev_big[:])

                # batched cascade levels across all pred tiles:
                # [npt, w] -> [npt, w/2] in-place (out == first operand),
                # then one batched reduce over the final [npt, 128] slices
                w = HW2 // 2
                v = h2[:, :].rearrange("p (a b) -> p a b", a=npt)
                while w >= 128:
                    nc.vector.tensor_tensor(
                        v[:, :, 0:w], v[:, :, 0:w], v[:, :, w : 2 * w], op=MAX
                    )
                    w //= 2
                nc.vector.tensor_reduce(
                    p2g_min[:], v[:, :, 0:128], axis=X, op=MAX
                )

                # ---- column (gt2pred) finish: GPSIMD partition all-reduce
                # (runs off the DVE; overlaps the next pass's main loop via
                # the double-buffered colacc)
                nc.gpsimd.partition_all_reduce(
                    g2p_all[:], colacc[:], 128, bass_isa.ReduceOp.max
                )

                nc.sync.dma_start(o_all[:, :], p2g_min[:])
                nc.sync.dma_start(g2p_out[:, :], g2p_all[0:1, :])

    nc.compile()
    return nc


def get_nc(v1=V1, v2c=V2C, repeat=1, variant="v3"):
    key = (v1, v2c, repeat, variant)
    if key not in _BUILT:
        _BUILT[key] = _build_v3(v1, v2c, repeat)
    return _BUILT[key]


def make_aug(gt, xp):
    """Fused augmented matmul operand [A | G]: one K=5 matmul yields the
    full squared-distance expansion |xp|^2 + |gt|^2 - 2 xp.gt."""
    v2c = xp.shape[0]
    ag = np.empty((5, v2c + gt.shape[0]), np.float32)
    ag[0:3, :v2c] = -2.0 * xp.T
    ag[3, :v2c] = (xp * xp).sum(-1)
    ag[4, :v2c] = 1.0
    ag[0:3, v2c:] = gt.T
    ag[3, v2c:] = 1.0
    ag[4, v2c:] = (gt * gt).sum(-1)
    return ag


def make_aug15(gt, xp):
    """K=15 packed hi/lo fp16 operand: rows 0-4 hi.hi, 5-9 A_lo vs G_hi,
    10-14 A_hi vs G_lo (the lo.lo term is dropped, ~2^-22 relative)."""
    v2c = xp.shape[0]
    ag = make_aug(gt, xp)
    ag[:, :v2c] *= -1.0  # negated A side -> matmul yields -d2 (max-fold scheme)
    hi = ag.astype(np.float16)
    lo = (ag - hi.astype(np.float32)).astype(np.float16)
    ag15 = np.empty((15, ag.shape[1]), np.float16)
    ag15[0:5] = hi
    ag15[5:10, :v2c] = lo[:, :v2c]
    ag15[5:10, v2c:] = hi[:, v2c:]
    ag15[10:15, :v2c] = hi[:, :v2c]
    ag15[10:15, v2c:] = lo[:, v2c:]
    return ag15


def plan_compaction(mask):
    """Per-core kept-pred indices and the common padded tile count."""
    kept = []
    for c in range(N_CORES):
        b, s = divmod(c, SLICES)
        sl = slice(s * V2C, (s + 1) * V2C)
        idx = np.nonzero(mask[b, sl] > 0.5)[0]
        kept.append((b, s * V2C, idx))
    max_kept = max(len(idx) for _, _, idx in kept)
    npt_eff = max(1, -(-max_kept // 128))
    return kept, npt_eff * 128


def make_in_maps(x_gt, x_pred, mask, confidence=None):
    """Shard full inputs into per-core input maps (host-side layout only).
    Masked preds are compacted out; padding rows are the origin point,
    which is idempotent for gt2pred (masked preds already sit there)."""
    kept, v2c_eff = plan_compaction(mask)
    in_maps = []
    for c in range(N_CORES):
        b, off, idx = kept[c]
        xp = np.zeros((v2c_eff, 3), np.float32)
        xp[: len(idx)] = x_pred[b, off + idx]
        in_maps.append({"ag": make_aug15(x_gt[b], xp)})
    return in_maps, kept, v2c_eff


def assemble_outputs(results, kept, v2c_eff, mask, confidence):
    """Host epilogue: sqrt/scale/weight raw -d2 device outputs and scatter
    kept-pred results back to their original positions."""
    npt = v2c_eff // 128
    loss_conf = np.zeros((B, V2), dtype=np.float32)
    loss_p2g = np.zeros((B, V2), dtype=np.float32)
    g2p_neg = np.full((B, V1), -np.inf, dtype=np.float32)
    for c in range(N_CORES):
        b, off, idx = kept[c]
        o = results[c]["o_all"]  # [128, npt] raw -d2 row maxima
        rows = o[:, :npt].T.reshape(v2c_eff)[: len(idx)]
        L = 100.0 * np.sqrt(np.maximum(-rows, 0.0))
        cf = confidence[b, off + idx]
        loss_p2g[b, off + idx] = L
        loss_conf[b, off + idx] = L * cf - np.log(cf)
        np.maximum(g2p_neg[b], results[c]["g2p"].T.reshape(V1), out=g2p_neg[b])
    loss_g2p = 100.0 * np.sqrt(np.maximum(-g2p_neg, 0.0))
    return loss_conf, loss_p2g, loss_g2p


def kernel(x_gt, x_pred, mask, confidence):
    from concourse.bass_utils import run_bass_kernel_spmd

    x_gt = np.asarray(x_gt)
    x_pred = np.asarray(x_pred)
    mask = np.asarray(mask)
    confidence = np.asarray(confidence)
    in_maps, kept, v2c_eff = make_in_maps(x_gt, x_pred, mask)
    nc = get_nc(v2c=v2c_eff)
    res = run_bass_kernel_spmd(nc, in_maps, list(range(N_CORES)))
    return assemble_outputs(res.results, kept, v2c_eff, mask, confidence)


# revision 24
# speedup vs baseline: 1.1558x; 1.1558x over previous
"""Bidirectional chamfer loss kernel for Trainium2 (8 NeuronCores).

Problem (hardcoded): B=2 batches, V1=8192 gt points, V2=8192 pred points, 3D.
  d2[b,i,j] = max(0, |xp_i|^2 + |gt_j|^2 - 2 xp_i.gt_j),  xp = x_pred * mask
  loss_pred2gt[b,i] = sqrt(min_j d2) * 100
  loss_gt2pred[b,j] = sqrt(min_i d2) * 100
  loss_conf = (loss_pred2gt * conf - ln(conf)) * mask ; loss_pred2gt *= mask

Sharding: 8 cores = 2 batches x 4 V2-slices (2048 preds/core vs full 8192 gt).
Each core computes row mins (pred2gt) for its pred slice exactly, and a
partial col min (gt2pred) over its preds; the host combines partials with
np.maximum on -d2 (exact).

Host-side compaction: masked preds collapse to the origin and their
pred2gt outputs are zeroed by the mask anyway, so the host keeps only
unmasked preds (plus origin padding, which is idempotent for gt2pred --
every slice retains its masked-at-origin points) and pads to a multiple
of 128. For ~80% keep rate this drops npt from 16 to 13 tiles.

Device kernel (per core, SPMD):
  PE matmul cost is N moving columns regardless of contraction depth K<=128,
  so the fp16 hi/lo split (A_hi.G_hi + A_lo.G_hi + A_hi.G_lo) is packed
  into ONE K=15 matmul -- fp32-grade d2 at fp16 matmul cost. The A side is
  negated so the matmul yields -d2 and every fold is a MAX.

  Per pred tile (128 preds x full 8192 gt): 16 N=512 matmuls into 4
  [128,2048] PSUM tiles; ScalarE downconverts each once into one
  contiguous [128,8192] fp16 buffer (1 elem/cycle/lane, the drain floor).
  DVE then runs the fewest possible fp16 2x-mode TTs: ONE full-width
  column fold into colacc (pair-seeded at pt==1, so no seed copy), and
  two per-pt row halving levels (8192->4096->2048) into a slot of a
  [128, npt*2048] buffer; the remaining halving levels run BATCHED
  across all pred tiles as in-place 3D-AP TTs, ending in one batched
  [128,npt,128] TensorReduce. Both paths sit at the DVE structural
  floor (12 col TTs = information-theoretic minimum for 13 tiles; the
  cascade consumes 4 fp16/cycle/lane with batch-amortized init cost).

  gt2pred finish: gpsimd.partition_all_reduce(max) on the double-buffered
  colacc -- runs on the otherwise-idle GPSIMD and overlaps the next
  pass's main loop, costing the DVE nothing (the former PE-transpose +
  DVE-reduce tail cost ~10us of DVE).

  The device returns RAW -d2 row/col maxima; sqrt, *100, mask/confidence
  weighting, ln(conf), and scatter back to original pred positions all
  happen on the host (cheap numpy on 16K values) -- no activations on
  device at all, so no activation-table loads.

  The `repeat` build parameter wraps the ENTIRE body (input DMA, main
  loop, transpose finish, output DMA) so the work-scaling timing harness
  measures the full per-pass device time.
"""

import numpy as np

B = 2
V1 = 8192  # gt points
V2 = 8192  # pred points (total)
N_CORES = 8
SLICES = N_CORES // B  # V2-slices per batch
V2C = V2 // SLICES  # pred points per core

_BUILT = {}


def _build_v3(v1, v2c, repeat=1, mmw=512, serial=False):
    import concourse.tile as tile
    from concourse import bacc, bass_isa, mybir

    f32 = mybir.dt.float32
    f16 = mybir.dt.float16
    MAX = mybir.AluOpType.max
    X = mybir.AxisListType.X

    npt = v2c // 128  # pred tiles
    W = min(2048, v1)  # gt group width: one PSUM tile, one ScalarE downconvert
    ng = v1 // W  # gt groups
    ngt = v1 // 128  # gt output tiles (transpose finish)
    nq = W // 512  # [128,512] transpose-output tiles per group
    S = v2c + v1

    nc = bacc.Bacc()
    ag_in = nc.dram_tensor("ag", [15, S], f16, kind="ExternalInput")
    o_all = nc.dram_tensor("o_all", [128, npt], f32, kind="ExternalOutput")
    g2p_out = nc.dram_tensor("g2p", [1, v1], f16, kind="ExternalOutput")

    CW = ng * W  # full gt width (8192): one col accumulator, one col TT
    HW2 = CW // 4  # per-pt cascade handoff width (2048)

    with tile.TileContext(nc) as tc:
        with (
            tc.tile_pool(name="persist", bufs=1) as P,
            tc.tile_pool(name="s16p", bufs=3) as S16P,
            tc.tile_pool(name="hp", bufs=2) as HP,
            tc.tile_pool(name="colp", bufs=2) as COLP,
            tc.tile_pool(name="mmps", bufs=2, space="PSUM") as MMPS,
        ):
            AG = P.tile([15, S], f16, tag="AG")
            A = AG[:, 0:v2c]
            G = AG[:, v2c:S]
            h2 = P.tile([128, npt * HW2], f16, tag="h2")
            p2g_min = P.tile([128, npt], f32, tag="p2gmin")
            g2p_all = P.tile([128, CW], f16, tag="g2pall")

            for r in range(repeat):
                colacc = COLP.tile([128, CW], f16, tag="colacc")
                if serial and r > 0:
                    # serialize pass boundaries through the all-reduce so
                    # the repeat marginal measures SINGLE-DISPATCH latency
                    nc.vector.tensor_copy(AG[0:1, 0:1], g2p_all[0:1, 0:1])
                # split input DMA: [A | G group 0] first so the first
                # matmul can start ~3.5us into the pass instead of ~8us
                c1 = v2c + W
                nc.sync.dma_start(AG[:, 0:c1], ag_in[:, 0:c1])
                if c1 < S:
                    nc.sync.dma_start(AG[:, c1:S], ag_in[:, c1:S])

                # ---- main loop ----
                # All 4 gt-group PSUM tiles drain into one contiguous
                # [128, 8192] fp16 buffer, so the column fold is ONE
                # full-width TT and the row path is a halving cascade of
                # full-width TTs -- fewest possible DVE instructions.
                prev_big = None
                for pt in range(npt):
                    lhsT = A[:, pt * 128 : (pt + 1) * 128]
                    big = S16P.tile([128, CW], f16, tag="s16")
                    for g in range(ng):
                        ps = MMPS.tile([128, W], f32, tag="mm")
                        for i in range(W // mmw):
                            nc.tensor.matmul(
                                ps[:, i * mmw : (i + 1) * mmw],
                                lhsT,
                                G[:, g * W + i * mmw : g * W + (i + 1) * mmw],
                                start=True,
                                stop=True,
                            )
                        nc.scalar.copy(big[:, g * W : (g + 1) * W], ps[:])
                    # column fold: pair-seed at pt==1, plain fold after
                    if pt == 1:
                        nc.vector.tensor_tensor(
                            colacc[:], prev_big[:], big[:], op=MAX
                        )
                    elif pt > 1:
                        nc.vector.tensor_tensor(
                            colacc[:], colacc[:], big[:], op=MAX
                        )
                    prev_big = big
                    # row path: two halving levels per pred tile
                    # (CW -> CW/2 -> CW/4 into this pt's h2 slot); the
                    # remaining levels run batched across all pred tiles
                    # after the loop (fewer DVE instructions, same work).
                    h = HP.tile([128, CW // 2], f16, tag="h")
                    nc.vector.tensor_tensor(
                        h[:], big[:, 0 : CW // 2], big[:, CW // 2 : CW], op=MAX
                    )
                    nc.vector.tensor_tensor(
                        h2[:, pt * HW2 : pt * HW2 + HW2],
                        h[:, 0:HW2],
                        h[:, HW2 : 2 * HW2],
                        op=MAX,
                    )
                if npt == 1:
                    nc.vector.tensor_copy(colacc[:], prev_big[:])

                # batched cascade levels across all pred tiles:
                # [npt, w] -> [npt, w/2] in-place (out == first operand),
                # then one batched reduce over the final [npt, 128] slices
                w = HW2 // 2
                v = h2[:, :].rearrange("p (a b) -> p a b", a=npt)
                while w >= 128:
                    nc.vector.tensor_tensor(
                        v[:, :, 0:w], v[:, :, 0:w], v[:, :, w : 2 * w], op=MAX
                    )
                    w //= 2
                nc.vector.tensor_reduce(
                    p2g_min[:], v[:, :, 0:128], axis=X, op=MAX
                )

                # ---- column (gt2pred) finish: GPSIMD partition all-reduce
                # (runs off the DVE; overlaps the next pass's main loop via
                # the double-buffered colacc)
                nc.gpsimd.partition_all_reduce(
                    g2p_all[:], colacc[:], 128, bass_isa.ReduceOp.max
                )

                nc.sync.dma_start(o_all[:, :], p2g_min[:])
                nc.sync.dma_start(g2p_out[:, :], g2p_all[0:1, :])

    nc.compile()
    return nc


def get_nc(v1=V1, v2c=V2C, repeat=1, variant="v3", serial=False):
    key = (v1, v2c, repeat, variant, serial)
    if key not in _BUILT:
        _BUILT[key] = _build_v3(v1, v2c, repeat, serial=serial)
    return _BUILT[key]


def make_aug(gt, xp):
    """Fused augmented matmul operand [A | G]: one K=5 matmul yields the
    full squared-distance expansion |xp|^2 + |gt|^2 - 2 xp.gt."""
    v2c = xp.shape[0]
    ag = np.empty((5, v2c + gt.shape[0]), np.float32)
    ag[0:3, :v2c] = -2.0 * xp.T
    ag[3, :v2c] = (xp * xp).sum(-1)
    ag[4, :v2c] = 1.0
    ag[0:3, v2c:] = gt.T
    ag[3, v2c:] = 1.0
    ag[4, v2c:] = (gt * gt).sum(-1)
    return ag


def make_aug15(gt, xp):
    """K=15 packed hi/lo fp16 operand: rows 0-4 hi.hi, 5-9 A_lo vs G_hi,
    10-14 A_hi vs G_lo (the lo.lo term is dropped, ~2^-22 relative)."""
    v2c = xp.shape[0]
    ag = make_aug(gt, xp)
    ag[:, :v2c] *= -1.0  # negated A side -> matmul yields -d2 (max-fold scheme)
    hi = ag.astype(np.float16)
    lo = (ag - hi.astype(np.float32)).astype(np.float16)
    ag15 = np.empty((15, ag.shape[1]), np.float16)
    ag15[0:5] = hi
    ag15[5:10, :v2c] = lo[:, :v2c]
    ag15[5:10, v2c:] = hi[:, v2c:]
    ag15[10:15, :v2c] = hi[:, :v2c]
    ag15[10:15, v2c:] = lo[:, v2c:]
    return ag15


def plan_compaction(mask):
    """Per-core kept-pred indices and the common padded tile count."""
    kept = []
    for c in range(N_CORES):
        b, s = divmod(c, SLICES)
        sl = slice(s * V2C, (s + 1) * V2C)
        idx = np.nonzero(mask[b, sl] > 0.5)[0]
        kept.append((b, s * V2C, idx))
    max_kept = max(len(idx) for _, _, idx in kept)
    npt_eff = max(1, -(-max_kept // 128))
    return kept, npt_eff * 128


def make_in_maps(x_gt, x_pred, mask, confidence=None):
    """Shard full inputs into per-core input maps (host-side layout only).
    Masked preds are compacted out; padding rows are the origin point,
    which is idempotent for gt2pred (masked preds already sit there)."""
    kept, v2c_eff = plan_compaction(mask)
    in_maps = []
    for c in range(N_CORES):
        b, off, idx = kept[c]
        xp = np.zeros((v2c_eff, 3), np.float32)
        xp[: len(idx)] = x_pred[b, off + idx]
        in_maps.append({"ag": make_aug15(x_gt[b], xp)})
    return in_maps, kept, v2c_eff


def assemble_outputs(results, kept, v2c_eff, mask, confidence):
    """Host epilogue: sqrt/scale/weight raw -d2 device outputs and scatter
    kept-pred results back to their original positions."""
    npt = v2c_eff // 128
    loss_conf = np.zeros((B, V2), dtype=np.float32)
    loss_p2g = np.zeros((B, V2), dtype=np.float32)
    g2p_neg = np.full((B, V1), -np.inf, dtype=np.float32)
    for c in range(N_CORES):
        b, off, idx = kept[c]
        o = results[c]["o_all"]  # [128, npt] raw -d2 row maxima
        rows = o[:, :npt].T.reshape(v2c_eff)[: len(idx)]
        L = 100.0 * np.sqrt(np.maximum(-rows, 0.0))
        cf = confidence[b, off + idx]
        loss_p2g[b, off + idx] = L
        loss_conf[b, off + idx] = L * cf - np.log(cf)
        np.maximum(g2p_neg[b], results[c]["g2p"].T.reshape(V1), out=g2p_neg[b])
    loss_g2p = 100.0 * np.sqrt(np.maximum(-g2p_neg, 0.0))
    return loss_conf, loss_p2g, loss_g2p


def kernel(x_gt, x_pred, mask, confidence):
    from concourse.bass_utils import run_bass_kernel_spmd

    x_gt = np.asarray(x_gt)
    x_pred = np.asarray(x_pred)
    mask = np.asarray(mask)
    confidence = np.asarray(confidence)
    in_maps, kept, v2c_eff = make_in_maps(x_gt, x_pred, mask)
    nc = get_nc(v2c=v2c_eff)
    res = run_bass_kernel_spmd(nc, in_maps, list(range(N_CORES)))
    return assemble_outputs(res.results, kept, v2c_eff, mask, confidence)


# revision 26
# speedup vs baseline: 1.1570x; 1.0010x over previous
"""Bidirectional chamfer loss kernel for Trainium2 (8 NeuronCores).

Problem (hardcoded): B=2 batches, V1=8192 gt points, V2=8192 pred points, 3D.
  d2[b,i,j] = max(0, |xp_i|^2 + |gt_j|^2 - 2 xp_i.gt_j),  xp = x_pred * mask
  loss_pred2gt[b,i] = sqrt(min_j d2) * 100
  loss_gt2pred[b,j] = sqrt(min_i d2) * 100
  loss_conf = (loss_pred2gt * conf - ln(conf)) * mask ; loss_pred2gt *= mask

Sharding: 8 cores = 2 batches x 4 V2-slices (2048 preds/core vs full 8192 gt).
Each core computes row mins (pred2gt) for its pred slice exactly, and a
partial col min (gt2pred) over its preds; the host combines partials with
np.maximum on -d2 (exact).

Host-side compaction: masked preds collapse to the origin and their
pred2gt outputs are zeroed by the mask anyway, so the host keeps only
unmasked preds (plus origin padding, which is idempotent for gt2pred --
every slice retains its masked-at-origin points) and pads to a multiple
of 128. For ~80% keep rate this drops npt from 16 to 13 tiles.

Device kernel (per core, SPMD):
  PE matmul cost is N moving columns regardless of contraction depth K<=128,
  so the fp16 hi/lo split (A_hi.G_hi + A_lo.G_hi + A_hi.G_lo) is packed
  into ONE K=15 matmul -- fp32-grade d2 at fp16 matmul cost. The A side is
  negated so the matmul yields -d2 and every fold is a MAX.

  Per pred tile (128 preds x full 8192 gt): 16 N=512 matmuls into 4
  [128,2048] PSUM tiles; ScalarE downconverts each once into one
  contiguous [128,8192] fp16 buffer (1 elem/cycle/lane, the drain floor).
  DVE then runs the fewest possible fp16 2x-mode TTs: ONE full-width
  column fold into colacc (pair-seeded at pt==1, so no seed copy), and
  two per-pt row halving levels (8192->4096->2048) into a slot of a
  [128, npt*2048] buffer; the remaining halving levels run BATCHED
  across all pred tiles as in-place 3D-AP TTs, ending in one batched
  [128,npt,128] TensorReduce. Both paths sit at the DVE structural
  floor (12 col TTs = information-theoretic minimum for 13 tiles; the
  cascade consumes 4 fp16/cycle/lane with batch-amortized init cost).

  gt2pred finish: gpsimd.partition_all_reduce(max) on the double-buffered
  colacc -- runs on the otherwise-idle GPSIMD and overlaps the next
  pass's main loop, costing the DVE nothing (the former PE-transpose +
  DVE-reduce tail cost ~10us of DVE).

  The device returns RAW -d2 row/col maxima; sqrt, *100, mask/confidence
  weighting, ln(conf), and scatter back to original pred positions all
  happen on the host (cheap numpy on 16K values) -- no activations on
  device at all, so no activation-table loads.

  The `repeat` build parameter wraps the ENTIRE body (input DMA, main
  loop, transpose finish, output DMA) so the work-scaling timing harness
  measures the full per-pass device time.
"""

import numpy as np

B = 2
V1 = 8192  # gt points
V2 = 8192  # pred points (total)
N_CORES = 8
SLICES = N_CORES // B  # V2-slices per batch
V2C = V2 // SLICES  # pred points per core

_BUILT = {}


def _build_v3(v1, v2c, repeat=1, mmw=512, serial=False, arc=1):
    import concourse.tile as tile
    from concourse import bacc, bass_isa, mybir

    f32 = mybir.dt.float32
    f16 = mybir.dt.float16
    MAX = mybir.AluOpType.max
    X = mybir.AxisListType.X

    npt = v2c // 128  # pred tiles
    W = min(2048, v1)  # gt group width: one PSUM tile, one ScalarE downconvert
    ng = v1 // W  # gt groups
    ngt = v1 // 128  # gt output tiles (transpose finish)
    nq = W // 512  # [128,512] transpose-output tiles per group
    S = v2c + v1

    nc = bacc.Bacc()
    ag_in = nc.dram_tensor("ag", [15, S], f16, kind="ExternalInput")
    o_all = nc.dram_tensor("o_all", [128, npt], f32, kind="ExternalOutput")
    g2p_out = nc.dram_tensor("g2p", [1, v1], f16, kind="ExternalOutput")

    CW = ng * W  # full gt width (8192): one col accumulator, one col TT
    HW2 = CW // 4  # per-pt cascade handoff width (2048)

    with tile.TileContext(nc) as tc:
        with (
            tc.tile_pool(name="persist", bufs=1) as P,
            tc.tile_pool(name="s16p", bufs=3) as S16P,
            tc.tile_pool(name="hp", bufs=2) as HP,
            tc.tile_pool(name="colp", bufs=2) as COLP,
            tc.tile_pool(name="mmps", bufs=2, space="PSUM") as MMPS,
        ):
            AG = P.tile([15, S], f16, tag="AG")
            A = AG[:, 0:v2c]
            G = AG[:, v2c:S]
            h2 = P.tile([128, npt * HW2], f16, tag="h2")
            p2g_min = P.tile([128, npt], f32, tag="p2gmin")
            g2p_all = P.tile([128, CW], f16, tag="g2pall")

            for r in range(repeat):
                colacc = COLP.tile([128, CW], f16, tag="colacc")
                if serial and r > 0:
                    # serialize pass boundaries through the all-reduce so
                    # the repeat marginal measures SINGLE-DISPATCH latency
                    nc.vector.tensor_copy(AG[0:1, 0:1], g2p_all[0:1, 0:1])
                # split input DMA: [A | G group 0] first so the first
                # matmul can start ~3.5us into the pass instead of ~8us
                c1 = v2c + W
                nc.sync.dma_start(AG[:, 0:c1], ag_in[:, 0:c1])
                if c1 < S:
                    nc.sync.dma_start(AG[:, c1:S], ag_in[:, c1:S])

                # ---- main loop ----
                # All 4 gt-group PSUM tiles drain into one contiguous
                # [128, 8192] fp16 buffer, so the column fold is ONE
                # full-width TT and the row path is a halving cascade of
                # full-width TTs -- fewest possible DVE instructions.
                prev_big = None
                for pt in range(npt):
                    lhsT = A[:, pt * 128 : (pt + 1) * 128]
                    big = S16P.tile([128, CW], f16, tag="s16")
                    for g in range(ng):
                        ps = MMPS.tile([128, W], f32, tag="mm")
                        for i in range(W // mmw):
                            nc.tensor.matmul(
                                ps[:, i * mmw : (i + 1) * mmw],
                                lhsT,
                                G[:, g * W + i * mmw : g * W + (i + 1) * mmw],
                                start=True,
                                stop=True,
                            )
                        nc.scalar.copy(big[:, g * W : (g + 1) * W], ps[:])
                    # column fold: pair-seed at pt==1, plain fold after
                    if pt == 1:
                        nc.vector.tensor_tensor(
                            colacc[:], prev_big[:], big[:], op=MAX
                        )
                    elif pt > 1:
                        nc.vector.tensor_tensor(
                            colacc[:], colacc[:], big[:], op=MAX
                        )
                    prev_big = big
                    # row path: two halving levels per pred tile
                    # (CW -> CW/2 -> CW/4 into this pt's h2 slot); the
                    # remaining levels run batched across all pred tiles
                    # after the loop (fewer DVE instructions, same work).
                    h = HP.tile([128, CW // 2], f16, tag="h")
                    nc.vector.tensor_tensor(
                        h[:], big[:, 0 : CW // 2], big[:, CW // 2 : CW], op=MAX
                    )
                    nc.vector.tensor_tensor(
                        h2[:, pt * HW2 : pt * HW2 + HW2],
                        h[:, 0:HW2],
                        h[:, HW2 : 2 * HW2],
                        op=MAX,
                    )
                if npt == 1:
                    nc.vector.tensor_copy(colacc[:], prev_big[:])

                # batched cascade levels across all pred tiles:
                # [npt, w] -> [npt, w/2] in-place (out == first operand),
                # then one batched reduce over the final [npt, 128] slices
                w = HW2 // 2
                v = h2[:, :].rearrange("p (a b) -> p a b", a=npt)
                while w >= 128:
                    nc.vector.tensor_tensor(
                        v[:, :, 0:w], v[:, :, 0:w], v[:, :, w : 2 * w], op=MAX
                    )
                    w //= 2
                nc.vector.tensor_reduce(
                    p2g_min[:], v[:, :, 0:128], axis=X, op=MAX
                )

                # ---- column (gt2pred) finish: GPSIMD partition all-reduce
                # (runs off the DVE; overlaps the next pass's main loop via
                # the double-buffered colacc)
                cwc = CW // arc
                for a in range(arc):
                    nc.gpsimd.partition_all_reduce(
                        g2p_all[:, a * cwc : (a + 1) * cwc],
                        colacc[:, a * cwc : (a + 1) * cwc],
                        128,
                        bass_isa.ReduceOp.max,
                    )

                nc.sync.dma_start(o_all[:, :], p2g_min[:])
                nc.sync.dma_start(g2p_out[:, :], g2p_all[0:1, :])

    nc.compile()
    return nc


def get_nc(v1=V1, v2c=V2C, repeat=1, variant="v3", serial=False, arc=1):
    key = (v1, v2c, repeat, variant, serial, arc)
    if key not in _BUILT:
        _BUILT[key] = _build_v3(v1, v2c, repeat, serial=serial, arc=arc)
    return _BUILT[key]


def make_aug(gt, xp):
    """Fused augmented matmul operand [A | G]: one K=5 matmul yields the
    full squared-distance expansion |xp|^2 + |gt|^2 - 2 xp.gt."""
    v2c = xp.shape[0]
    ag = np.empty((5, v2c + gt.shape[0]), np.float32)
    ag[0:3, :v2c] = -2.0 * xp.T
    ag[3, :v2c] = (xp * xp).sum(-1)
    ag[4, :v2c] = 1.0
    ag[0:3, v2c:] = gt.T
    ag[3, v2c:] = 1.0
    ag[4, v2c:] = (gt * gt).sum(-1)
    return ag


def make_aug15(gt, xp):
    """K=15 packed hi/lo fp16 operand: rows 0-4 hi.hi, 5-9 A_lo vs G_hi,
    10-14 A_hi vs G_lo (the lo.lo term is dropped, ~2^-22 relative)."""
    v2c = xp.shape[0]
    ag = make_aug(gt, xp)
    ag[:, :v2c] *= -1.0  # negated A side -> matmul yields -d2 (max-fold scheme)
    hi = ag.astype(np.float16)
    lo = (ag - hi.astype(np.float32)).astype(np.float16)
    ag15 = np.empty((15, ag.shape[1]), np.float16)
    ag15[0:5] = hi
    ag15[5:10, :v2c] = lo[:, :v2c]
    ag15[5:10, v2c:] = hi[:, v2c:]
    ag15[10:15, :v2c] = hi[:, :v2c]
    ag15[10:15, v2c:] = lo[:, v2c:]
    return ag15


def plan_compaction(mask):
    """Per-core kept-pred indices and the common padded tile count."""
    kept = []
    for c in range(N_CORES):
        b, s = divmod(c, SLICES)
        sl = slice(s * V2C, (s + 1) * V2C)
        idx = np.nonzero(mask[b, sl] > 0.5)[0]
        kept.append((b, s * V2C, idx))
    max_kept = max(len(idx) for _, _, idx in kept)
    npt_eff = max(1, -(-max_kept // 128))
    return kept, npt_eff * 128


def make_in_maps(x_gt, x_pred, mask, confidence=None):
    """Shard full inputs into per-core input maps (host-side layout only).
    Masked preds are compacted out; padding rows are the origin point,
    which is idempotent for gt2pred (masked preds already sit there)."""
    kept, v2c_eff = plan_compaction(mask)
    in_maps = []
    for c in range(N_CORES):
        b, off, idx = kept[c]
        xp = np.zeros((v2c_eff, 3), np.float32)
        xp[: len(idx)] = x_pred[b, off + idx]
        in_maps.append({"ag": make_aug15(x_gt[b], xp)})
    return in_maps, kept, v2c_eff


def assemble_outputs(results, kept, v2c_eff, mask, confidence):
    """Host epilogue: sqrt/scale/weight raw -d2 device outputs and scatter
    kept-pred results back to their original positions."""
    npt = v2c_eff // 128
    loss_conf = np.zeros((B, V2), dtype=np.float32)
    loss_p2g = np.zeros((B, V2), dtype=np.float32)
    g2p_neg = np.full((B, V1), -np.inf, dtype=np.float32)
    for c in range(N_CORES):
        b, off, idx = kept[c]
        o = results[c]["o_all"]  # [128, npt] raw -d2 row maxima
        rows = o[:, :npt].T.reshape(v2c_eff)[: len(idx)]
        L = 100.0 * np.sqrt(np.maximum(-rows, 0.0))
        cf = confidence[b, off + idx]
        loss_p2g[b, off + idx] = L
        loss_conf[b, off + idx] = L * cf - np.log(cf)
        np.maximum(g2p_neg[b], results[c]["g2p"].T.reshape(V1), out=g2p_neg[b])
    loss_g2p = 100.0 * np.sqrt(np.maximum(-g2p_neg, 0.0))
    return loss_conf, loss_p2g, loss_g2p


def kernel(x_gt, x_pred, mask, confidence):
    from concourse.bass_utils import run_bass_kernel_spmd

    x_gt = np.asarray(x_gt)
    x_pred = np.asarray(x_pred)
    mask = np.asarray(mask)
    confidence = np.asarray(confidence)
    in_maps, kept, v2c_eff = make_in_maps(x_gt, x_pred, mask)
    nc = get_nc(v2c=v2c_eff)
    res = run_bass_kernel_spmd(nc, in_maps, list(range(N_CORES)))
    return assemble_outputs(res.results, kept, v2c_eff, mask, confidence)


# revision 28
# speedup vs baseline: 1.2023x; 1.0392x over previous
"""Bidirectional chamfer loss kernel for Trainium2 (8 NeuronCores).

Problem (hardcoded): B=2 batches, V1=8192 gt points, V2=8192 pred points, 3D.
  d2[b,i,j] = max(0, |xp_i|^2 + |gt_j|^2 - 2 xp_i.gt_j),  xp = x_pred * mask
  loss_pred2gt[b,i] = sqrt(min_j d2) * 100
  loss_gt2pred[b,j] = sqrt(min_i d2) * 100
  loss_conf = (loss_pred2gt * conf - ln(conf)) * mask ; loss_pred2gt *= mask

Sharding: 8 cores = 2 batches x 4 V2-slices (2048 preds/core vs full 8192 gt).
Each core computes row mins (pred2gt) for its pred slice exactly, and a
partial col min (gt2pred) over its preds; the host combines partials with
np.maximum on -d2 (exact).

Host-side compaction: masked preds collapse to the origin and their
pred2gt outputs are zeroed by the mask anyway, so the host keeps only
unmasked preds (plus origin padding, which is idempotent for gt2pred --
every slice retains its masked-at-origin points) and pads to a multiple
of 128. For ~80% keep rate this drops npt from 16 to 13 tiles.

Device kernel (per core, SPMD):
  PE matmul cost is N moving columns regardless of contraction depth K<=128,
  so the fp16 hi/lo split (A_hi.G_hi + A_lo.G_hi + A_hi.G_lo) is packed
  into ONE K=15 matmul -- fp32-grade d2 at fp16 matmul cost. The A side is
  negated so the matmul yields -d2 and every fold is a MAX.

  Per pred tile (128 preds x full 8192 gt): 16 N=512 matmuls into 4
  [128,2048] PSUM tiles; ScalarE downconverts each once into one
  contiguous [128,8192] fp16 buffer (1 elem/cycle/lane, the drain floor).
  DVE then runs the fewest possible fp16 2x-mode TTs: ONE full-width
  column fold into colacc (pair-seeded at pt==1, so no seed copy), and
  two per-pt row halving levels (8192->4096->2048) into a slot of a
  [128, npt*2048] buffer; the remaining halving levels run BATCHED
  across all pred tiles as in-place 3D-AP TTs, ending in one batched
  [128,npt,128] TensorReduce. Both paths sit at the DVE structural
  floor (12 col TTs = information-theoretic minimum for 13 tiles; the
  cascade consumes 4 fp16/cycle/lane with batch-amortized init cost).

  gt2pred finish: gpsimd.partition_all_reduce(max) on the double-buffered
  colacc -- runs on the otherwise-idle GPSIMD and overlaps the next
  pass's main loop, costing the DVE nothing (the former PE-transpose +
  DVE-reduce tail cost ~10us of DVE).

  The device returns RAW -d2 row/col maxima; sqrt, *100, mask/confidence
  weighting, ln(conf), and scatter back to original pred positions all
  happen on the host (cheap numpy on 16K values) -- no activations on
  device at all, so no activation-table loads.

  The `repeat` build parameter wraps the ENTIRE body (input DMA, main
  loop, transpose finish, output DMA) so the work-scaling timing harness
  measures the full per-pass device time.
"""

import numpy as np

B = 2
V1 = 8192  # gt points
V2 = 8192  # pred points (total)
N_CORES = 8
SLICES = N_CORES // B  # V2-slices per batch
V2C = V2 // SLICES  # pred points per core

_BUILT = {}


def _build_v3(v1, v2c, repeat=1, mmw=512, serial=False, arc=1):
    import concourse.tile as tile
    from concourse import bacc, bass_isa, mybir

    f32 = mybir.dt.float32
    f16 = mybir.dt.float16
    MAX = mybir.AluOpType.max
    X = mybir.AxisListType.X

    npt = v2c // 128  # pred tiles
    W = min(2048, v1)  # gt group width: one PSUM tile, one ScalarE downconvert
    ng = v1 // W  # gt groups
    ngt = v1 // 128  # gt output tiles (transpose finish)
    nq = W // 512  # [128,512] transpose-output tiles per group
    S = v2c + v1

    nc = bacc.Bacc()
    ag_in = nc.dram_tensor("ag", [15, S], f16, kind="ExternalInput")
    o_all = nc.dram_tensor("o_all", [128, npt], f32, kind="ExternalOutput")
    g2p_out = nc.dram_tensor("g2p", [1, v1], f16, kind="ExternalOutput")

    CW = ng * W  # full gt width (8192): one col accumulator, one col TT
    HW2 = CW // 4  # per-pt cascade handoff width (2048)

    with tile.TileContext(nc) as tc:
        with (
            tc.tile_pool(name="persist", bufs=1) as P,
            tc.tile_pool(name="s16p", bufs=3) as S16P,
            tc.tile_pool(name="hp", bufs=2) as HP,
            tc.tile_pool(name="colp", bufs=2) as COLP,
            tc.tile_pool(name="mmps", bufs=2, space="PSUM") as MMPS,
        ):
            AG = P.tile([15, S], f16, tag="AG")
            A = AG[:, 0:v2c]
            G = AG[:, v2c:S]
            h2 = P.tile([128, npt * HW2], f16, tag="h2")
            p2g_min = P.tile([128, npt], f32, tag="p2gmin")
            g2p_all = P.tile([128, CW], f16, tag="g2pall")

            for r in range(repeat):
                colacc = COLP.tile([128, CW], f16, tag="colacc")
                if serial and r > 0:
                    # serialize pass boundaries through the all-reduce so
                    # the repeat marginal measures SINGLE-DISPATCH latency
                    nc.vector.tensor_copy(AG[0:1, 0:1], g2p_all[0:1, 0:1])
                # split input DMA: [A | G group 0] first so the first
                # matmul can start ~3.5us into the pass instead of ~8us
                c1 = v2c + W
                nc.sync.dma_start(AG[:, 0:c1], ag_in[:, 0:c1])
                if c1 < S:
                    nc.sync.dma_start(AG[:, c1:S], ag_in[:, c1:S])

                # ---- main loop ----
                # All 4 gt-group PSUM tiles drain into one contiguous
                # [128, 8192] fp16 buffer, so the column fold is ONE
                # full-width TT and the row path is a halving cascade of
                # full-width TTs -- fewest possible DVE instructions.
                prev_big = None
                for pt in range(npt):
                    lhsT = A[:, pt * 128 : (pt + 1) * 128]
                    big = S16P.tile([128, CW], f16, tag="s16")
                    for g in range(ng):
                        ps = MMPS.tile([128, W], f32, tag="mm")
                        for i in range(W // mmw):
                            nc.tensor.matmul(
                                ps[:, i * mmw : (i + 1) * mmw],
                                lhsT,
                                G[:, g * W + i * mmw : g * W + (i + 1) * mmw],
                                start=True,
                                stop=True,
                            )
                        nc.scalar.copy(big[:, g * W : (g + 1) * W], ps[:])
                    # column fold: pair-seed at pt==1, plain fold after
                    if pt == 1:
                        nc.vector.tensor_tensor(
                            colacc[:], prev_big[:], big[:], op=MAX
                        )
                    elif pt > 1:
                        nc.vector.tensor_tensor(
                            colacc[:], colacc[:], big[:], op=MAX
                        )
                    prev_big = big
                    # row path: two halving levels per pred tile
                    # (CW -> CW/2 -> CW/4 into this pt's h2 slot); the
                    # remaining levels run batched across all pred tiles
                    # after the loop (fewer DVE instructions, same work).
                    h = HP.tile([128, CW // 2], f16, tag="h")
                    nc.vector.tensor_tensor(
                        h[:], big[:, 0 : CW // 2], big[:, CW // 2 : CW], op=MAX
                    )
                    nc.vector.tensor_tensor(
                        h2[:, pt * HW2 : pt * HW2 + HW2],
                        h[:, 0:HW2],
                        h[:, HW2 : 2 * HW2],
                        op=MAX,
                    )
                if npt == 1:
                    nc.vector.tensor_copy(colacc[:], prev_big[:])

                # batched cascade levels across all pred tiles:
                # [npt, w] -> [npt, w/2] in-place (out == first operand),
                # then one batched reduce over the final [npt, 128] slices
                w = HW2 // 2
                v = h2[:, :].rearrange("p (a b) -> p a b", a=npt)
                while w >= 128:
                    nc.vector.tensor_tensor(
                        v[:, :, 0:w], v[:, :, 0:w], v[:, :, w : 2 * w], op=MAX
                    )
                    w //= 2
                nc.vector.tensor_reduce(
                    p2g_min[:], v[:, :, 0:128], axis=X, op=MAX
                )

                # ---- column (gt2pred) finish: GPSIMD partition all-reduce
                # (runs off the DVE; overlaps the next pass's main loop via
                # the double-buffered colacc)
                cwc = CW // arc
                for a in range(arc):
                    nc.gpsimd.partition_all_reduce(
                        g2p_all[:, a * cwc : (a + 1) * cwc],
                        colacc[:, a * cwc : (a + 1) * cwc],
                        128,
                        bass_isa.ReduceOp.max,
                    )

                nc.sync.dma_start(o_all[:, :], p2g_min[:])
                nc.sync.dma_start(g2p_out[:, :], g2p_all[0:1, :])

    nc.compile()
    return nc


def get_nc(v1=V1, v2c=V2C, repeat=1, variant="v3", serial=False, arc=1):
    key = (v1, v2c, repeat, variant, serial, arc)
    if key not in _BUILT:
        _BUILT[key] = _build_v3(v1, v2c, repeat, serial=serial, arc=arc)
    return _BUILT[key]


def make_aug(gt, xp):
    """Fused augmented matmul operand [A | G]: one K=5 matmul yields the
    full squared-distance expansion |xp|^2 + |gt|^2 - 2 xp.gt."""
    v2c = xp.shape[0]
    ag = np.empty((5, v2c + gt.shape[0]), np.float32)
    ag[0:3, :v2c] = -2.0 * xp.T
    ag[3, :v2c] = (xp * xp).sum(-1)
    ag[4, :v2c] = 1.0
    ag[0:3, v2c:] = gt.T
    ag[3, v2c:] = 1.0
    ag[4, v2c:] = (gt * gt).sum(-1)
    return ag


def make_aug15(gt, xp):
    """K=15 packed hi/lo fp16 operand: rows 0-4 hi.hi, 5-9 A_lo vs G_hi,
    10-14 A_hi vs G_lo (the lo.lo term is dropped, ~2^-22 relative)."""
    v2c = xp.shape[0]
    ag = make_aug(gt, xp)
    ag[:, :v2c] *= -1.0  # negated A side -> matmul yields -d2 (max-fold scheme)
    hi = ag.astype(np.float16)
    lo = (ag - hi.astype(np.float32)).astype(np.float16)
    ag15 = np.empty((15, ag.shape[1]), np.float16)
    ag15[0:5] = hi
    ag15[5:10, :v2c] = lo[:, :v2c]
    ag15[5:10, v2c:] = hi[:, v2c:]
    ag15[10:15, :v2c] = hi[:, :v2c]
    ag15[10:15, v2c:] = lo[:, v2c:]
    return ag15


def plan_compaction(mask):
    """Per-core kept-pred indices and the common padded tile count."""
    kept = []
    for c in range(N_CORES):
        b, s = divmod(c, SLICES)
        sl = slice(s * V2C, (s + 1) * V2C)
        idx = np.nonzero(mask[b, sl] > 0.5)[0]
        kept.append((b, s * V2C, idx))
    max_kept = max(len(idx) for _, _, idx in kept)
    npt_eff = max(1, -(-max_kept // 128))
    return kept, npt_eff * 128


def make_in_maps(x_gt, x_pred, mask, confidence=None):
    """Shard full inputs into per-core input maps (host-side layout only).
    Masked preds are compacted out; padding rows are the origin point,
    which is idempotent for gt2pred (masked preds already sit there)."""
    kept, v2c_eff = plan_compaction(mask)
    in_maps = []
    for c in range(N_CORES):
        b, off, idx = kept[c]
        xp = np.zeros((v2c_eff, 3), np.float32)
        xp[: len(idx)] = x_pred[b, off + idx]
        in_maps.append({"ag": make_aug15(x_gt[b], xp)})
    return in_maps, kept, v2c_eff


def assemble_outputs(results, kept, v2c_eff, mask, confidence):
    """Host epilogue: sqrt/scale/weight raw -d2 device outputs and scatter
    kept-pred results back to their original positions."""
    npt = v2c_eff // 128
    loss_conf = np.zeros((B, V2), dtype=np.float32)
    loss_p2g = np.zeros((B, V2), dtype=np.float32)
    g2p_neg = np.full((B, V1), -np.inf, dtype=np.float32)
    for c in range(N_CORES):
        b, off, idx = kept[c]
        o = results[c]["o_all"]  # [128, npt] raw -d2 row maxima
        rows = o[:, :npt].T.reshape(v2c_eff)[: len(idx)]
        L = 100.0 * np.sqrt(np.maximum(-rows, 0.0))
        cf = confidence[b, off + idx]
        loss_p2g[b, off + idx] = L
        loss_conf[b, off + idx] = L * cf - np.log(cf)
        np.maximum(g2p_neg[b], results[c]["g2p"].T.reshape(V1), out=g2p_neg[b])
    loss_g2p = 100.0 * np.sqrt(np.maximum(-g2p_neg, 0.0))
    return loss_conf, loss_p2g, loss_g2p


def kernel(x_gt, x_pred, mask, confidence):
    from concourse.bass_utils import run_bass_kernel_spmd

    x_gt = np.asarray(x_gt)
    x_pred = np.asarray(x_pred)
    mask = np.asarray(mask)
    confidence = np.asarray(confidence)
    in_maps, kept, v2c_eff = make_in_maps(x_gt, x_pred, mask)
    nc = get_nc(v2c=v2c_eff)
    res = run_bass_kernel_spmd(nc, in_maps, list(range(N_CORES)))
    return assemble_outputs(res.results, kept, v2c_eff, mask, confidence)
